# revision 17
# baseline (speedup 1.0000x reference)
"""Self-contained Trainium2 Bass kernel for nn_AttentionHead_89687507076307.

Problem: single-head causal attention, B=8, S=2048, D_IN=1024, D_OUT=64, fp32.
Sharding: data-parallel over batch -- each of the 8 NeuronCores computes one
batch element end to end; no collectives.

Host marshaling (part of input sharding): X tensors are transposed to
[D, S] layout and cast to bf16 per core; weights are cast to bf16 and split
into 128-row d-chunks.  The device then runs a pure-bf16 matmul pipeline
(PE native rate) with no on-device transposes of X:

  kT/qT/vT [64,S]: W-chunk stationary, X^T pumped  (contract d, PSUM accum)
  vaug  [k,65]   : PE transpose of vT tiles + ones col (natural [S,64]+sums)
  scoresT [k,q]  : kT-tile stationary, qT pumped    (contract e=64)
  expT           : ACT exp(0.125 * scores) -> bf16, causal quarter-mask on
                   diagonal tiles; fully-masked columns skipped
  av [65, q]     : vaug stationary, expT pumped     (contract k; row 64=sums)

Loads are pipelined at [128,512] column-chunk granularity so query-block qb
attends while qb+1 data streams in.  Device emits unnormalized av tiles
[4, 65, 512] fp32; the host divides by the sums row and transposes back to
[S, 64] during the gather/unshard step.
"""
import sys

for _p in ("/opt/trn_rl_repo",):
    if _p not in sys.path:
        sys.path.append(_p)

from contextlib import ExitStack

import numpy as np

import concourse.bass as bass
import concourse.mybir as mybir
import concourse.tile as tile
from concourse import bacc

B, S, D, E = 8, 2048, 1024, 64
SB = 512               # q block size
NSB = S // SB          # 4
NKT = S // 128         # 16 k-tiles
NDC = D // 128         # 8 d-chunks
F32 = mybir.dt.float32
BF16 = mybir.dt.bfloat16
EXP = mybir.ActivationFunctionType.Exp
N_CORES = 8


def build_nc():
    nc = bacc.Bacc("TRN2", target_bir_lowering=False, debug=False)

    xkT_d = nc.dram_tensor("xkT", [NSB, 128, NDC * SB], BF16, kind="ExternalInput").ap()
    xqT_d = nc.dram_tensor("xqT", [NSB, 128, NDC * SB], BF16, kind="ExternalInput").ap()
    xvT_d = nc.dram_tensor("xvT", [NSB, 128, NDC * SB], BF16, kind="ExternalInput").ap()
    w_d = nc.dram_tensor("wkqv", [128, 3 * NDC * E], BF16, kind="ExternalInput").ap()
    mask_d = nc.dram_tensor("mask", [128, 128], BF16, kind="ExternalInput").ap()
    ident_d = nc.dram_tensor("ident64", [64, 64], BF16, kind="ExternalInput").ap()
    av_d = nc.dram_tensor("avout", [NSB, 65, SB], F32, kind="ExternalOutput").ap()

    with tile.TileContext(nc) as tc, ExitStack() as ctx:
        const = ctx.enter_context(tc.tile_pool(name="const", bufs=1))
        wall = const.tile([128, 3, NDC, E], BF16, name="wkqv")
        nc.sync.dma_start(out=wall[:], in_=w_d.rearrange("p (t c e) -> p t c e", t=3, e=E))
        w_tiles = {"wk": wall[:, 0], "wq": wall[:, 1], "wv": wall[:, 2]}
        mask = const.tile([128, 128], BF16, name="mask")
        nc.scalar.dma_start(out=mask[:], in_=mask_d[:])
        ident64 = const.tile([64, 64], BF16, name="ident64")
        nc.scalar.dma_start(out=ident64[:], in_=ident_d[:])

        # X^T column blocks: one tile per (tensor, sb) so the load pipeline
        # has no false WAR hazards against compute reading earlier blocks
        xk_sb = [const.tile([128, NDC, SB], BF16, name=f"xk{s}") for s in range(NSB)]
        xq_sb = [const.tile([128, NDC, SB], BF16, name=f"xq{s}") for s in range(NSB)]
        xv_sb = [const.tile([128, NDC, SB], BF16, name=f"xv{s}") for s in range(NSB)]

        kT = const.tile([E, S], BF16, name="kT")
        qT = const.tile([E, S], BF16, name="qT")
        vT = const.tile([E, S], BF16, name="vT")
        vaug = const.tile([128, NKT, 65], BF16, name="vaug")
        nc.gpsimd.memset(vaug[:, :, E : E + 1], 1.0)

        pk_pool = ctx.enter_context(tc.tile_pool(name="pk", bufs=1, space="PSUM"))
        pv_pool = ctx.enter_context(tc.tile_pool(name="pv", bufs=1, space="PSUM"))
        sc_pool = ctx.enter_context(tc.tile_pool(name="sc", bufs=2, space="PSUM"))
        av_pool = ctx.enter_context(tc.tile_pool(name="av", bufs=2, space="PSUM"))
        exp_pool = ctx.enter_context(tc.tile_pool(name="exp", bufs=8))
        osb_pool = ctx.enter_context(tc.tile_pool(name="osb", bufs=2))

        # column-block load pipeline: one 1MB DMA per (tensor, sb), all on
        # the dedicated sync queue in need-order (8KB rows -> ~500GB/s)
        def load_sb(sb):
            nc.sync.dma_start(
                out=xk_sb[sb][:].rearrange("p c s -> p (c s)"), in_=xkT_d[sb]
            )
            nc.sync.dma_start(
                out=xq_sb[sb][:].rearrange("p c s -> p (c s)"), in_=xqT_d[sb]
            )
            nc.sync.dma_start(
                out=xv_sb[sb][:].rearrange("p c s -> p (c s)"), in_=xvT_d[sb]
            )

        warm = const.tile([128, SB], BF16, name="warm")
        nc.vector.memset(warm[:], 0.0)
        wpj = pk_pool.tile([E, SB], F32, name="pj0")
        for _ in range(64):
            nc.tensor.matmul(
                wpj[:], lhsT=warm[:, 0:E], rhs=warm[:], start=True, stop=True
            )

        ncopy = 0

        def proj3(sb):
            # k/q/v projections interleaved across three PSUM banks so
            # back-to-back accumulations never target the same bank
            nonlocal ncopy
            pjs = [pk_pool.tile([E, SB], F32, name=f"pj{t}") for t in range(3)]
            srcs = [
                (w_tiles["wk"], xk_sb[sb], kT),
                (w_tiles["wq"], xq_sb[sb], qT),
                (w_tiles["wv"], xv_sb[sb], vT),
            ]
            for dc in range(NDC):
                for t, (wt, xt, _) in enumerate(srcs):
                    nc.tensor.matmul(
                        pjs[t][:],
                        lhsT=wt[:, dc, :],
                        rhs=xt[:, dc, :],
                        start=(dc == 0),
                        stop=(dc == NDC - 1),
                    )
            for t, (_, _, dst) in enumerate(srcs):
                nc.vector.tensor_copy(dst[:, sb * SB : (sb + 1) * SB], pjs[t][:])

        def vfill(sb):
            # vaug[:, kt, 0:64] = vT[:, kt-tile].T via PE bf16 transpose
            for kt in range(4 * sb, 4 * sb + 4):
                pvt = pv_pool.tile([128, E], BF16, name="pvt")
                nc.tensor.transpose(
                    pvt[:], vT[:, kt * 128 : (kt + 1) * 128], ident64[:]
                )
                nc.vector.tensor_copy(vaug[:, kt, 0:E], pvt[:])

        def attention(qb):
            n_kt = 4 * qb + 4
            avp = av_pool.tile([65, SB], F32, name="avp")

            def scores(kt):
                j = kt - 4 * qb  # >= 0 -> diagonal band tile
                c0 = j * 128 if j > 0 else 0
                scp = sc_pool.tile([128, SB], F32, name="scp")
                nc.tensor.matmul(
                    scp[:, c0:],
                    lhsT=kT[:, kt * 128 : (kt + 1) * 128],
                    rhs=qT[:, qb * SB + c0 : (qb + 1) * SB],
                    start=True,
                    stop=True,
                )
                et = exp_pool.tile([128, SB], BF16, name="et")
                nc.scalar.activation(et[:, c0:], scp[:, c0:], EXP, scale=0.125)
                if j >= 0:
                    nc.vector.tensor_mul(
                        et[:, j * 128 : (j + 1) * 128],
                        et[:, j * 128 : (j + 1) * 128],
                        mask[:],
                    )
                return et, c0

            def av(kt, et, c0):
                nc.tensor.matmul(
                    avp[:, c0:],
                    lhsT=vaug[:, kt, :],
                    rhs=et[:, c0:],
                    start=(kt == 0),
                    stop=(kt == n_kt - 1),
                    skip_group_check=True,
                )

            pend = []
            for kt in range(n_kt):
                pend.append((kt,) + scores(kt))
                if len(pend) > 1:
                    av(*pend.pop(0))
            while pend:
                av(*pend.pop(0))
            osb = osb_pool.tile([65, SB], F32, name="osb")
            nc.vector.tensor_copy(osb[:], avp[:])
            nc.sync.dma_start(out=av_d[qb], in_=osb[:])

        # software pipeline: per sb, stream loads, project k/q/v, attend block
        for sb in range(NSB):
            load_sb(sb)
        for sb in range(NSB):
            if sb > 0:
                for _ in range(2):
                    nc.tensor.matmul(
                        wpj[:], lhsT=warm[:, 0:E], rhs=warm[:], start=True, stop=True
                    )
            proj3(sb)
            vfill(sb)
            attention(sb)

    nc.compile()
    return nc


_NC = None


def _get_nc():
    global _NC
    if _NC is None:
        _NC = build_nc()
    return _NC


def _in_maps(inputs):
    import ml_dtypes

    bf16 = ml_dtypes.bfloat16
    def wprep(w):
        # [1024, 64] -> [128, NDC*E], d = dc*128 + p
        return np.ascontiguousarray(
            np.asarray(w, np.float32).reshape(NDC, 128, E).transpose(1, 0, 2)
        ).reshape(128, NDC * E).astype(bf16)

    wkqv = np.ascontiguousarray(
        np.concatenate(
            [wprep(inputs["K"]), wprep(inputs["Q"]), wprep(inputs["V"])], axis=1
        )
    )
    mask = np.triu(np.ones((128, 128), np.float32)).astype(bf16)
    ident64 = np.eye(64, dtype=np.float32).astype(bf16)
    xk = np.asarray(inputs["inputs_for_keys"], np.float32)
    xq = np.asarray(inputs["inputs_for_queries"], np.float32)
    xv = np.asarray(inputs["inputs_for_values"], np.float32)
    def xprep(x):
        # [S, D] -> X^T [NSB, 128, NDC*SB]: xT[sb, p, dc*SB + s] =
        # x[sb*SB + s, dc*128 + p]
        xT = x.T.astype(bf16)                       # [D, S] = [(dc p), (sb s)]
        xT = xT.reshape(NDC, 128, NSB, SB)
        return np.ascontiguousarray(xT.transpose(2, 1, 0, 3)).reshape(
            NSB, 128, NDC * SB
        )

    maps = []
    for b in range(N_CORES):
        m = {
            "xkT": xprep(xk[b]),
            "xqT": xprep(xq[b]),
            "xvT": xprep(xv[b]),
            "wkqv": wkqv,
            "mask": mask,
            "ident64": ident64,
        }
        maps.append(m)
    return maps


def _post(res):
    out = np.empty((N_CORES, S, E), np.float32)
    for b in range(N_CORES):
        av = np.asarray(res.results[b]["avout"], np.float32)  # [NSB, 65, SB]
        num = av[:, :E, :]                                    # [NSB, 64, SB]
        den = av[:, E : E + 1, :]                             # [NSB, 1, SB]
        o = num / den                                         # [NSB, 64, SB]
        out[b] = o.transpose(0, 2, 1).reshape(S, E)
    return out


def kernel(**inputs):
    from concourse.bass_utils import run_bass_kernel_spmd

    nc = _get_nc()
    res = run_bass_kernel_spmd(nc, _in_maps(inputs), core_ids=list(range(N_CORES)))
    return np.ascontiguousarray(_post(res))


def kernel_profiled(**inputs):
    """Like kernel() but with neuron-profile NTFF capture (dev/test use only)."""
    import types

    from trn_agent_boot.trn_boot import _ntff_profile_via_ctypes

    hook = _ntff_profile_via_ctypes("/opt/axon/libaxon_pjrt.so")
    m = types.ModuleType("antenv.axon_hooks")
    m.get_axon_ntff_profile_hook = lambda: hook
    m.set_axon_ntff_profile_hook = lambda h: None
    sys.modules["antenv.axon_hooks"] = m

    from concourse import bass_utils

    bass_utils.upload_artifacts = lambda tmpdir: tmpdir

    nc = _get_nc()
    res = bass_utils.run_bass_kernel_spmd(
        nc,
        _in_maps(inputs),
        core_ids=list(range(N_CORES)),
        trace=True,
        tmpdir="/tmp/attn_trace",
    )
    return np.ascontiguousarray(_post(res)), res


# revision 18
# speedup vs baseline: 1.1145x; 1.1145x over previous
"""Self-contained Trainium2 Bass kernel for nn_AttentionHead_89687507076307.

Problem: single-head causal attention, B=8, S=2048, D_IN=1024, D_OUT=64, fp32.
Sharding: data-parallel over batch -- each of the 8 NeuronCores computes one
batch element end to end; no collectives.

Host marshaling (part of input sharding): X tensors are transposed to
[D, S] layout and cast to bf16 per core; weights are cast to bf16 and split
into 128-row d-chunks.  The device then runs a pure-bf16 matmul pipeline
(PE native rate) with no on-device transposes of X:

  kT/qT/vT [64,S]: W-chunk stationary, X^T pumped  (contract d, PSUM accum)
  vaug  [k,65]   : PE transpose of vT tiles + ones col (natural [S,64]+sums)
  scoresT [k,q]  : kT-tile stationary, qT pumped    (contract e=64)
  expT           : ACT exp(0.125 * scores) -> bf16, causal quarter-mask on
                   diagonal tiles; fully-masked columns skipped
  av [65, q]     : vaug stationary, expT pumped     (contract k; row 64=sums)

Loads are pipelined at [128,512] column-chunk granularity so query-block qb
attends while qb+1 data streams in.  Device emits unnormalized av tiles
[4, 65, 512] fp32; the host divides by the sums row and transposes back to
[S, 64] during the gather/unshard step.
"""
import sys

for _p in ("/opt/trn_rl_repo",):
    if _p not in sys.path:
        sys.path.append(_p)

from contextlib import ExitStack

import numpy as np

import concourse.bass as bass
import concourse.mybir as mybir
import concourse.tile as tile
from concourse import bacc

B, S, D, E = 8, 2048, 1024, 64
SB = 512               # q block size
NSB = S // SB          # 4
NKT = S // 128         # 16 k-tiles
NDC = D // 128         # 8 d-chunks
F32 = mybir.dt.float32
BF16 = mybir.dt.bfloat16
EXP = mybir.ActivationFunctionType.Exp
N_CORES = 8


def build_nc():
    nc = bacc.Bacc("TRN2", target_bir_lowering=False, debug=False)

    xkT_d = nc.dram_tensor("xkT", [NSB, 128, NDC * SB], BF16, kind="ExternalInput").ap()
    xqT_d = nc.dram_tensor("xqT", [NSB, 128, NDC * SB], BF16, kind="ExternalInput").ap()
    xvT_d = nc.dram_tensor("xvT", [NSB, 128, NDC * SB], BF16, kind="ExternalInput").ap()
    w_d = nc.dram_tensor("wkqv", [128, 3 * NDC * E], BF16, kind="ExternalInput").ap()
    mask_d = nc.dram_tensor("mask", [128, 128], BF16, kind="ExternalInput").ap()
    ident_d = nc.dram_tensor("ident64", [64, 64], BF16, kind="ExternalInput").ap()
    av_d = nc.dram_tensor("avout", [NSB, 65, SB], F32, kind="ExternalOutput").ap()

    with tile.TileContext(nc) as tc, ExitStack() as ctx:
        const = ctx.enter_context(tc.tile_pool(name="const", bufs=1))
        wall = const.tile([128, 3, NDC, E], BF16, name="wkqv")
        nc.sync.dma_start(out=wall[:], in_=w_d.rearrange("p (t c e) -> p t c e", t=3, e=E))
        w_tiles = {"wk": wall[:, 0], "wq": wall[:, 1], "wv": wall[:, 2]}
        mask = const.tile([128, 128], BF16, name="mask")
        nc.scalar.dma_start(out=mask[:], in_=mask_d[:])
        ident64 = const.tile([64, 64], BF16, name="ident64")
        nc.scalar.dma_start(out=ident64[:], in_=ident_d[:])

        # X^T column blocks: one tile per (tensor, sb) so the load pipeline
        # has no false WAR hazards against compute reading earlier blocks
        xk_sb = [const.tile([128, NDC, SB], BF16, name=f"xk{s}") for s in range(NSB)]
        xq_sb = [const.tile([128, NDC, SB], BF16, name=f"xq{s}") for s in range(NSB)]
        xv_sb = [const.tile([128, NDC, SB], BF16, name=f"xv{s}") for s in range(NSB)]

        kT = const.tile([E, S], BF16, name="kT")
        qT = const.tile([E, S], BF16, name="qT")
        vT = const.tile([E, S], BF16, name="vT")
        vaug = const.tile([128, NKT, 65], BF16, name="vaug")
        nc.gpsimd.memset(vaug[:, :, E : E + 1], 1.0)

        pk_pool = ctx.enter_context(tc.tile_pool(name="pk", bufs=1, space="PSUM"))
        pv_pool = ctx.enter_context(tc.tile_pool(name="pv", bufs=1, space="PSUM"))
        sc_pool = ctx.enter_context(tc.tile_pool(name="sc", bufs=2, space="PSUM"))
        av_pool = ctx.enter_context(tc.tile_pool(name="av", bufs=2, space="PSUM"))
        exp_pool = ctx.enter_context(tc.tile_pool(name="exp", bufs=8))
        osb_pool = ctx.enter_context(tc.tile_pool(name="osb", bufs=2))

        # column-block load pipeline: one 1MB DMA per (tensor, sb), all on
        # the dedicated sync queue in need-order (8KB rows -> ~500GB/s)
        def load_sb(sb):
            nc.sync.dma_start(
                out=xk_sb[sb][:].rearrange("p c s -> p (c s)"), in_=xkT_d[sb]
            )
            nc.sync.dma_start(
                out=xq_sb[sb][:].rearrange("p c s -> p (c s)"), in_=xqT_d[sb]
            )
            nc.sync.dma_start(
                out=xv_sb[sb][:].rearrange("p c s -> p (c s)"), in_=xvT_d[sb]
            )

        warm = const.tile([128, SB], BF16, name="warm")
        nc.vector.memset(warm[:], 0.0)
        wpj = pk_pool.tile([E, SB], F32, name="pj0")
        for _ in range(32):
            nc.tensor.matmul(
                wpj[:], lhsT=warm[:, 0:E], rhs=warm[:], start=True, stop=True
            )

        ncopy = 0

        def proj3(sb):
            # k/q/v projections interleaved across three PSUM banks so
            # back-to-back accumulations never target the same bank
            nonlocal ncopy
            pjs = [pk_pool.tile([E, SB], F32, name=f"pj{t}") for t in range(3)]
            srcs = [
                (w_tiles["wk"], xk_sb[sb], kT),
                (w_tiles["wq"], xq_sb[sb], qT),
                (w_tiles["wv"], xv_sb[sb], vT),
            ]
            for dc in range(NDC):
                for t, (wt, xt, _) in enumerate(srcs):
                    nc.tensor.matmul(
                        pjs[t][:],
                        lhsT=wt[:, dc, :],
                        rhs=xt[:, dc, :],
                        start=(dc == 0),
                        stop=(dc == NDC - 1),
                    )
            for t, (_, _, dst) in enumerate(srcs):
                nc.vector.tensor_copy(dst[:, sb * SB : (sb + 1) * SB], pjs[t][:])

        def vfill(sb):
            # vaug[:, kt, 0:64] = vT[:, kt-tile].T via PE bf16 transpose
            for kt in range(4 * sb, 4 * sb + 4):
                pvt = pv_pool.tile([128, E], BF16, name="pvt")
                nc.tensor.transpose(
                    pvt[:], vT[:, kt * 128 : (kt + 1) * 128], ident64[:]
                )
                nc.vector.tensor_copy(vaug[:, kt, 0:E], pvt[:])

        def attention(qb):
            n_kt = 4 * qb + 4
            avp = av_pool.tile([65, SB], F32, name="avp")

            def scores(kt):
                j = kt - 4 * qb  # >= 0 -> diagonal band tile
                c0 = j * 128 if j > 0 else 0
                scp = sc_pool.tile([128, SB], F32, name="scp")
                nc.tensor.matmul(
                    scp[:, c0:],
                    lhsT=kT[:, kt * 128 : (kt + 1) * 128],
                    rhs=qT[:, qb * SB + c0 : (qb + 1) * SB],
                    start=True,
                    stop=True,
                )
                et = exp_pool.tile([128, SB], BF16, name="et")
                nc.scalar.activation(et[:, c0:], scp[:, c0:], EXP, scale=0.125)
                if j >= 0:
                    nc.vector.tensor_mul(
                        et[:, j * 128 : (j + 1) * 128],
                        et[:, j * 128 : (j + 1) * 128],
                        mask[:],
                    )
                return et, c0

            def av(kt, et, c0):
                nc.tensor.matmul(
                    avp[:, c0:],
                    lhsT=vaug[:, kt, :],
                    rhs=et[:, c0:],
                    start=(kt == 0),
                    stop=(kt == n_kt - 1),
                    skip_group_check=True,
                )

            pend = []
            for kt in range(n_kt):
                pend.append((kt,) + scores(kt))
                if len(pend) > 1:
                    av(*pend.pop(0))
            while pend:
                av(*pend.pop(0))
            osb = osb_pool.tile([65, SB], F32, name="osb")
            nc.vector.tensor_copy(osb[:], avp[:])
            nc.sync.dma_start(out=av_d[qb], in_=osb[:])

        # software pipeline: per sb, stream loads, project k/q/v, attend block
        for sb in range(NSB):
            load_sb(sb)
        for sb in range(NSB):
            if sb > 0:
                for _ in range(2):
                    nc.tensor.matmul(
                        wpj[:], lhsT=warm[:, 0:E], rhs=warm[:], start=True, stop=True
                    )
            proj3(sb)
            vfill(sb)
            attention(sb)

    nc.compile()
    return nc


_NC = None


def _get_nc():
    global _NC
    if _NC is None:
        _NC = build_nc()
    return _NC


def _in_maps(inputs):
    import ml_dtypes

    bf16 = ml_dtypes.bfloat16
    def wprep(w):
        # [1024, 64] -> [128, NDC*E], d = dc*128 + p
        return np.ascontiguousarray(
            np.asarray(w, np.float32).reshape(NDC, 128, E).transpose(1, 0, 2)
        ).reshape(128, NDC * E).astype(bf16)

    wkqv = np.ascontiguousarray(
        np.concatenate(
            [wprep(inputs["K"]), wprep(inputs["Q"]), wprep(inputs["V"])], axis=1
        )
    )
    mask = np.triu(np.ones((128, 128), np.float32)).astype(bf16)
    ident64 = np.eye(64, dtype=np.float32).astype(bf16)
    xk = np.asarray(inputs["inputs_for_keys"], np.float32)
    xq = np.asarray(inputs["inputs_for_queries"], np.float32)
    xv = np.asarray(inputs["inputs_for_values"], np.float32)
    def xprep(x):
        # [S, D] -> X^T [NSB, 128, NDC*SB]: xT[sb, p, dc*SB + s] =
        # x[sb*SB + s, dc*128 + p]
        xT = x.T.astype(bf16)                       # [D, S] = [(dc p), (sb s)]
        xT = xT.reshape(NDC, 128, NSB, SB)
        return np.ascontiguousarray(xT.transpose(2, 1, 0, 3)).reshape(
            NSB, 128, NDC * SB
        )

    maps = []
    for b in range(N_CORES):
        m = {
            "xkT": xprep(xk[b]),
            "xqT": xprep(xq[b]),
            "xvT": xprep(xv[b]),
            "wkqv": wkqv,
            "mask": mask,
            "ident64": ident64,
        }
        maps.append(m)
    return maps


def _post(res):
    out = np.empty((N_CORES, S, E), np.float32)
    for b in range(N_CORES):
        av = np.asarray(res.results[b]["avout"], np.float32)  # [NSB, 65, SB]
        num = av[:, :E, :]                                    # [NSB, 64, SB]
        den = av[:, E : E + 1, :]                             # [NSB, 1, SB]
        o = num / den                                         # [NSB, 64, SB]
        out[b] = o.transpose(0, 2, 1).reshape(S, E)
    return out


def kernel(**inputs):
    from concourse.bass_utils import run_bass_kernel_spmd

    nc = _get_nc()
    res = run_bass_kernel_spmd(nc, _in_maps(inputs), core_ids=list(range(N_CORES)))
    return np.ascontiguousarray(_post(res))


def kernel_profiled(**inputs):
    """Like kernel() but with neuron-profile NTFF capture (dev/test use only)."""
    import types

    from trn_agent_boot.trn_boot import _ntff_profile_via_ctypes

    hook = _ntff_profile_via_ctypes("/opt/axon/libaxon_pjrt.so")
    m = types.ModuleType("antenv.axon_hooks")
    m.get_axon_ntff_profile_hook = lambda: hook
    m.set_axon_ntff_profile_hook = lambda h: None
    sys.modules["antenv.axon_hooks"] = m

    from concourse import bass_utils

    bass_utils.upload_artifacts = lambda tmpdir: tmpdir

    nc = _get_nc()
    res = bass_utils.run_bass_kernel_spmd(
        nc,
        _in_maps(inputs),
        core_ids=list(range(N_CORES)),
        trace=True,
        tmpdir="/tmp/attn_trace",
    )
    return np.ascontiguousarray(_post(res)), res
